# revision 1
# baseline (speedup 1.0000x reference)
"""ChannelPruner kernel for Trainium2 (8 NeuronCores, data-parallel over batch).

Math: out[b,o,h,w] = sum_c conv_weights[o,c,0,0] * x[b,c,h,w]   (1x1 conv).
For a ChannelPruner the weight is a diagonal matrix (identity with pruned
output channels zeroed), so out[b,c,h,w] = diag[c] * x[b,c,h,w] exactly.
We verify diagonality on the host at runtime and fall back to a dense GEMM
path if it ever isn't (it is, for this module).

Sharding: batch 32 -> 4 per core across 8 cores; the diag vector is
replicated. Each core streams [4, 256, 3136] f32 through SBUF with channels
on partitions (2 halves of 128), multiplies by a per-partition scalar on the
vector engine, and DMAs back out. This is HBM-bandwidth-bound.
"""

import numpy as np
from contextlib import ExitStack

import concourse.bass as bass
import concourse.bacc as bacc
import concourse.tile as tile
from concourse import mybir
from concourse.bass_utils import run_bass_kernel_spmd

B, C, H, W = 32, 256, 56, 56
F = H * W  # 3136
N_CORES = 8
BPC = B // N_CORES  # batches per core

_FP32 = mybir.dt.float32

_nc_cache = {}


def _build_scale_nc():
    """Per-core program: out[b,c,f] = diag[c] * x[b,c,f] for x [BPC, C, F]."""
    nc = bacc.Bacc("TRN2", target_bir_lowering=False, debug=False,
                   num_devices=N_CORES)
    x = nc.dram_tensor("x", [BPC, C, F], _FP32, kind="ExternalInput").ap()
    d = nc.dram_tensor("diag", [C, 1], _FP32, kind="ExternalInput").ap()
    o = nc.dram_tensor("out", [BPC, C, F], _FP32, kind="ExternalOutput").ap()

    with tile.TileContext(nc) as tc:
        with ExitStack() as ctx:
            dpool = ctx.enter_context(tc.tile_pool(name="diag", bufs=1))
            pool = ctx.enter_context(tc.tile_pool(name="data", bufs=6))

            dtiles = []
            for h in range(C // 128):
                dt_ = dpool.tile([128, 1], _FP32, tag=f"diag{h}")
                nc.sync.dma_start(dt_[:], d[h * 128:(h + 1) * 128, :])
                dtiles.append(dt_)

            for b in range(BPC):
                for h in range(C // 128):
                    t = pool.tile([128, F], _FP32)
                    nc.sync.dma_start(t[:], x[b, h * 128:(h + 1) * 128, :])
                    nc.vector.tensor_scalar_mul(t[:], t[:], dtiles[h][:])
                    nc.scalar.dma_start(o[b, h * 128:(h + 1) * 128, :], t[:])
    nc.compile()
    return nc


def kernel(x: np.ndarray, conv_weights: np.ndarray) -> np.ndarray:
    w = conv_weights[:, :, 0, 0].astype(np.float32)
    diag = np.ascontiguousarray(np.diagonal(w)).astype(np.float32)
    if not np.array_equal(np.diag(diag), w):
        # Non-diagonal weight: not a ChannelPruner instance; dense fallback.
        return np.einsum("bchw,oc->bohw", x, w).astype(x.dtype)

    if "scale" not in _nc_cache:
        _nc_cache["scale"] = _build_scale_nc()
    nc = _nc_cache["scale"]

    xr = np.ascontiguousarray(x.astype(np.float32)).reshape(B, C, F)
    diag_col = diag.reshape(C, 1)
    in_maps = [
        {"x": xr[i * BPC:(i + 1) * BPC], "diag": diag_col}
        for i in range(N_CORES)
    ]
    res = run_bass_kernel_spmd(nc, in_maps, list(range(N_CORES)))
    out = np.concatenate([r["out"] for r in res.results], axis=0)
    return out.reshape(B, C, H, W).astype(x.dtype)
